# revision 21
# baseline (speedup 1.0000x reference)
"""Trainium2 Bass kernel for ConcreteAttentionModel (dense_mlp).

Model (reference):
  img = relu(einsum('bnf,df->bnd', image_features, W_ic) + b_ic)   B=16,N=64,F=65536,D=512
  gated attention-MIL pooling over patches -> img [B, D]
  text = relu(text @ Wt.T + bt)                                    [B, D]
  h1 = relu(img @ Wh1.T + bh1)
  z1 = einsum('bi,oij,bj->bo', img, Wz1, text) + bz1               Wz1 [D,D,D]
  o1 = relu((sigmoid(z1) * h1) @ Wo1.T + bo1)
  fused = relu(kron(o1, text) @ Wpf.T + bpf)                       Wpf [D, D*D]
  out = fused @ Wcls.T + bcls                                      [B, 1]

Sharding over 8 cores (v2):
  - Stage 1 sharded over the contraction dim F (8192 per core); fp32 partial
    [D, B*N] AllReduce'd on-device; attention pooling replicated per core.
  - Bilinear Wz1 and post-fusion Wpf sharded over their output dim (64 rows
    per core).

v2 perf scheme:
  - The four big tensors (X, W_ic, Wz1, Wpf) are cast to fp8e4m3 on the host
    (X*16, weights*512; descale folded into activation scale / STT scalar).
    Per-core HBM traffic drops ~193 MB -> ~56 MB.  Numpy-validated rel err
    ~1.1e-2 vs the 2e-2 gate.  All activations are bf16; PSUM stays fp32.
  - Bilinears are col-tiled 4x via tile_position: col-group t computes
    o = g + 16t, quadrupling PE utilization (stationary has only B=16 cols).
    The per-group PSUM bank [128, D] is drained by ONE [128,512] DVE STT
    against a 4x-replicated textcT, accumulating z1/fused columns for 4 o's
    at once (partitions 32t+b).
  - All wz1 slab DMAs + a wpf prefetch window are emitted at the head of the
    SP DMA ring so they stream during stage 1 (HWDGE launches are FIFO per
    engine).  Aux DMAs (collective bounce, gate/out) ride the ACT ring.
"""

import numpy as np
import ml_dtypes

import concourse.bass as bass
import concourse.mybir as mybir
import concourse.tile as tile
from concourse import bacc
from concourse.masks import make_identity

F32 = mybir.dt.float32
F32R = mybir.dt.float32r
BF16 = mybir.dt.bfloat16
F8 = mybir.dt.float8e4

NP_BF16 = ml_dtypes.bfloat16
NP_F8 = ml_dtypes.float8_e4m3

NCORES = 8
B, N, F = 16, 64, 65536
BN = B * N                      # 1024
D, A, T = 512, 256, 18
FS = F // NCORES                # 8192 per-core contraction slice
KT = FS // 128                  # 64 k-tiles
KC = KT // 2                    # 32 k-chunks (2 k-tiles per DMA)
OS = D // NCORES                # 64 output rows per core (bilinear/post-fusion)
DT = D // 128                   # 4 partition tiles of the D dim
AT = A // 128                   # 2 partition tiles of the A dim
NG = OS // 4                    # 16 col-tiled bilinear groups (4 o's each)

SX = 16.0                       # image fp8 scale
SW = 512.0                      # weight fp8 scale
DESC1 = 1.0 / (SX * SW)         # stage-1 descale
DESCB = 1.0 / SW                # bilinear descale

WZ_PRE = 50                     # wz1 slabs prefetched at the DMA-ring head
WPF_PRE = 16                    # wpf slabs prefetched at the DMA-ring head


def build_nc(use_collectives: bool = True, reps: int = 1,
             doublerow: bool = True, ar_bf16: bool = True):
    nc = bacc.Bacc("TRN2", target_bir_lowering=False, debug=False,
                   num_devices=NCORES)

    def dram(name, shape, dt=F32):
        return nc.dram_tensor(name, shape, dt, kind="ExternalInput").ap()

    # per-core sharded inputs (fp8, host-prepacked for contiguous DMA lines)
    xT = dram("xT", [KC, 128, 2 * BN], F8)
    wicT = dram("wicT", [KC, 128, 2 * D], F8)
    wz1 = dram("wz1", [OS, 128, DT * D], F8)   # o-slab [128, it, j]
    wpf = dram("wpf", [OS, 128, DT * D], F8)   # m-slab [128, ot, j]
    wclsT = dram("wclsT", [OS, 1], BF16)
    bz1 = dram("bz1", [OS, 1])
    bpf = dram("bpf", [OS, 1])
    # replicated inputs
    textT = dram("textT", [T, B], BF16)
    waT = dram("waT", [D, A], BF16)
    wbT = dram("wbT", [D, A], BF16)
    wcT = dram("wcT", [A, 1], BF16)
    wtT = dram("wtT", [T, D], BF16)
    wh1T = dram("wh1T", [D, D], BF16)
    wo1T = dram("wo1T", [D, D], BF16)
    b_ic = dram("b_ic", [D, 1])
    ba = dram("ba", [A, 1])
    bb = dram("bb", [A, 1])
    bt = dram("bt", [D, 1])
    bh1 = dram("bh1", [D, 1])
    bo1 = dram("bo1", [D, 1])

    out_partial = nc.dram_tensor("out_partial", [1, B], F32,
                                 kind="ExternalOutput").ap()

    groups = [list(range(NCORES))]

    with tile.TileContext(nc) as tc:
        with (
            nc.allow_low_precision(reason="fp8/bf16 matmul operands"),
            tc.tile_pool(name="const", bufs=1) as cst,
            tc.tile_pool(name="xk", bufs=3) as xk,
            tc.tile_pool(name="wk", bufs=3) as wk,
            tc.tile_pool(name="wz", bufs=WZ_PRE) as wzp,
            tc.tile_pool(name="wpfp", bufs=WPF_PRE) as wpfp,
            tc.tile_pool(name="work", bufs=1) as wrk,
            tc.tile_pool(name="persist", bufs=1) as per,
            tc.tile_pool(name="ps", bufs=8, space="PSUM") as ps,
            tc.tile_pool(name="dramp", bufs=1, space="DRAM") as dp,
        ):
            # ---- constants / small weights -------------------------------
            ident = cst.tile([128, 128], F32, tag="ident", name="ident")
            make_identity(nc, ident)
            ones_f = cst.tile([1, 128], F32, tag="ones_f", name="ones_f")
            nc.gpsimd.memset(ones_f[:], 1.0)
            ones1 = cst.tile([1, 128], BF16, tag="ones1", name="ones1")
            nc.vector.tensor_copy(ones1[:], ones_f[:])

            def load_const(src, shape, tag, dt=F32):
                t = cst.tile(shape, dt, tag=tag, name=tag)
                nc.sync.dma_start(t[:], src)
                return t

            textT_sb = load_const(textT[:], [T, B], "textT", BF16)
            wtT_sb = load_const(wtT[:], [T, D], "wtT", BF16)
            waT_sb = [load_const(waT[k * 128:(k + 1) * 128, :], [128, A],
                                 f"waT{k}", BF16) for k in range(DT)]
            wbT_sb = [load_const(wbT[k * 128:(k + 1) * 128, :], [128, A],
                                 f"wbT{k}", BF16) for k in range(DT)]
            wcT_sb = [load_const(wcT[k * 128:(k + 1) * 128, :], [128, 1],
                                 f"wcT{k}", BF16) for k in range(AT)]
            wh1T_sb = [load_const(wh1T[k * 128:(k + 1) * 128, :], [128, D],
                                  f"wh1T{k}", BF16) for k in range(DT)]
            wo1T_sb = [load_const(wo1T[k * 128:(k + 1) * 128, :], [128, D],
                                  f"wo1T{k}", BF16) for k in range(DT)]
            wclsT_sb = [load_const(wclsT[NG * t:NG * (t + 1), :], [NG, 1],
                                   f"wclsT{t}", BF16) for t in range(4)]
            bic_sb = [load_const(b_ic[k * 128:(k + 1) * 128, :], [128, 1], f"bic{k}")
                      for k in range(DT)]
            ba_sb = [load_const(ba[k * 128:(k + 1) * 128, :], [128, 1], f"ba{k}")
                     for k in range(AT)]
            bb_sb = [load_const(bb[k * 128:(k + 1) * 128, :], [128, 1], f"bb{k}")
                     for k in range(AT)]
            bt_sb = [load_const(bt[k * 128:(k + 1) * 128, :], [128, 1], f"bt{k}")
                     for k in range(DT)]
            bh1_sb = [load_const(bh1[k * 128:(k + 1) * 128, :], [128, 1], f"bh1{k}")
                      for k in range(DT)]
            bo1_sb = [load_const(bo1[k * 128:(k + 1) * 128, :], [128, 1], f"bo1{k}")
                      for k in range(DT)]
            bz1_sb = [load_const(bz1[NG * t:NG * (t + 1), :], [NG, 1], f"bz1{t}")
                      for t in range(4)]
            bpf_sb = [load_const(bpf[NG * t:NG * (t + 1), :], [NG, 1], f"bpf{t}")
                      for t in range(4)]

            def emit_body():
                # ---- DMA-ring head: prefetch launches -------------------
                # consumption order for both bilinears: group g uses
                # o = g + 16t for t = 0..3
                order = [g + NG * t for g in range(NG) for t in range(4)]
                wz_slabs = {}
                for o in order[:WZ_PRE]:
                    t_ = wzp.tile([128, DT * D], F8, tag="wz", name=f"wz{o}")
                    nc.sync.dma_start(t_[:], wz1[o])
                    wz_slabs[o] = t_
                wpf_slabs = {}
                for o in order[:WPF_PRE]:
                    t_ = wpfp.tile([128, DT * D], F8, tag="wpf", name=f"wpf{o}")
                    nc.sync.dma_start(t_[:], wpf[o])
                    wpf_slabs[o] = t_

                # ---- text branch: textc = relu(Wt @ text.T + bt) [D, B] --
                textc = []
                for m in range(DT):
                    p = ps.tile([128, B], F32, tag="ps", name=f"tc_ps{m}")
                    nc.tensor.matmul(p[:], wtT_sb[:, m * 128:(m + 1) * 128],
                                     textT_sb[:], start=True, stop=True)
                    t = per.tile([128, B], F32, tag=f"textc{m}", name=f"textc{m}")
                    nc.scalar.activation(t[:], p[:],
                                         mybir.ActivationFunctionType.Relu,
                                         bias=bt_sb[m][:])
                    textc.append(t)
                # textcT4 [128, D]: textcT4[32t+b, j] = textc[j, b], 4x repl.
                textcT4 = per.tile([128, D], F32, tag="textcT4", name="textcT4")
                for m in range(DT):
                    p = ps.tile([B, 128], F32, tag="ps", name=f"tct_ps{m}")
                    nc.tensor.transpose(p[:], textc[m][:], ident[:, :])
                    for t in range(4):
                        nc.vector.tensor_copy(
                            textcT4[32 * t:32 * t + B, m * 128:(m + 1) * 128],
                            p[:])

                # ---- stage 1: partial img.T = (W_ic @ X.T) slice ---------
                s1ps = [ps.tile([128, 512], F32, tag="ps", name=f"s1ps{i}")
                        for i in range(8)]
                for c in range(KC):
                    xt = xk.tile([128, 2 * BN], F8, tag="xk", name=f"x{c}")
                    nc.sync.dma_start(xt[:], xT[c])
                    wt = wk.tile([128, 2 * D], F8, tag="wk", name=f"w{c}")
                    nc.sync.dma_start(wt[:], wicT[c])
                    # fp8 DoubleRow: contract both k-tiles of the chunk at once
                    xt3 = xt.rearrange("p (k n) -> p k n", k=2)
                    wt3 = wt.rearrange("p (k n) -> p k n", k=2)
                    if doublerow:
                        first = c == 0
                        last = c == KC - 1
                        for dt in range(DT):
                            for h in range(2):
                                nc.tensor.matmul(
                                    s1ps[dt * 2 + h][:],
                                    wt3[:, :, dt * 128:(dt + 1) * 128],
                                    xt3[:, :, h * 512:(h + 1) * 512],
                                    start=first, stop=last,
                                    perf_mode=mybir.MatmulPerfMode.DoubleRow)
                    else:
                        for k in range(2):
                            first = c == 0 and k == 0
                            last = c == KC - 1 and k == 1
                            for dt in range(DT):
                                for h in range(2):
                                    nc.tensor.matmul(
                                        s1ps[dt * 2 + h][:],
                                        wt3[:, k, dt * 128:(dt + 1) * 128],
                                        xt3[:, k, h * 512:(h + 1) * 512],
                                        start=first, stop=last)

                # partial -> DRAM bounce -> AllReduce -> img tiles
                shared_addr = "Shared" if use_collectives else "Local"
                ARDT = BF16 if ar_bf16 else F32
                ar_in = dp.tile([D, BN], ARDT, tag="ar_in", name="ar_in")
                ar_out = dp.tile([D, BN], ARDT, tag="ar_out", name="ar_out",
                                 addr_space=shared_addr)
                for dt in range(DT):
                    for h in range(2):
                        s = wrk.tile([128, 512], ARDT, tag="s1out", bufs=2,
                                     name=f"s1o{dt}{h}")
                        nc.vector.tensor_copy(s[:], s1ps[dt * 2 + h][:])
                        nc.scalar.dma_start(
                            ar_in[dt * 128:(dt + 1) * 128, h * 512:(h + 1) * 512],
                            s[:])
                if use_collectives:
                    nc.gpsimd.collective_compute(
                        "AllReduce", mybir.AluOpType.add, replica_groups=groups,
                        ins=[ar_in.opt()], outs=[ar_out.opt()])
                else:
                    nc.sync.dma_start(ar_out[:], ar_in[:])

                # img = relu(sum * DESC1 + b_ic): [D, BN] as 4 bf16 tiles
                img = []
                for dt in range(DT):
                    raw = wrk.tile([128, BN], ARDT, tag="imgraw", bufs=1,
                                   name=f"imgraw{dt}")
                    nc.scalar.dma_start(raw[:], ar_out[dt * 128:(dt + 1) * 128, :])
                    t = per.tile([128, BN], BF16, tag=f"img{dt}", name=f"img{dt}")
                    nc.scalar.activation(t[:], raw[:],
                                         mybir.ActivationFunctionType.Relu,
                                         bias=bic_sb[dt][:], scale=DESC1)
                    img.append(t)

                # ---- attention: a=tanh(Wa@img+ba), g=sig(Wb@img+bb) ------
                def attn_half(wT_sb, b_sb, func, nm):
                    outs = []
                    for m in range(AT):
                        t = wrk.tile([128, BN], BF16, tag=f"{nm}{m}", name=f"{nm}{m}")
                        for h in range(2):
                            p = ps.tile([128, 512], F32, tag="ps",
                                        name=f"{nm}_ps{m}{h}")
                            for k in range(DT):
                                nc.tensor.matmul(
                                    p[:],
                                    wT_sb[k][:, m * 128:(m + 1) * 128],
                                    img[k][:, h * 512:(h + 1) * 512],
                                    start=(k == 0), stop=(k == DT - 1))
                            nc.scalar.activation(t[:, h * 512:(h + 1) * 512], p[:],
                                                 func, bias=b_sb[m][:])
                        outs.append(t)
                    return outs

                a_sb = attn_half(waT_sb, ba_sb, mybir.ActivationFunctionType.Tanh, "a")
                g_sb = attn_half(wbT_sb, bb_sb, mybir.ActivationFunctionType.Sigmoid, "g")
                for m in range(AT):
                    nc.vector.tensor_mul(a_sb[m][:], a_sb[m][:], g_sb[m][:])

                # logits [1, BN] = Wc @ (a*g)
                sm = wrk.tile([1, BN], F32, tag="sm", name="sm")
                for h in range(2):
                    p = ps.tile([1, 512], F32, tag="ps", name=f"lg{h}")
                    for k in range(AT):
                        nc.tensor.matmul(p[:], wcT_sb[k][:],
                                         a_sb[k][:, h * 512:(h + 1) * 512],
                                         start=(k == 0), stop=(k == AT - 1))
                    nc.scalar.copy(sm[:, h * 512:(h + 1) * 512], p[:])

                # softmax over n (64) within each bag, * 1/N   -> wv [1, BN]
                smv = sm.rearrange("p (b n) -> p b n", n=N)
                mx = wrk.tile([1, B], F32, tag="mx", name="mx")
                nc.vector.tensor_reduce(mx[:], smv, mybir.AxisListType.X,
                                        mybir.AluOpType.max)
                ex = wrk.tile([1, BN], F32, tag="ex", name="ex")
                exv = ex.rearrange("p (b n) -> p b n", n=N)
                nc.vector.tensor_sub(exv, smv, mx[:, :, None].broadcast_to([1, B, N]))
                nc.scalar.activation(ex[:], ex[:], mybir.ActivationFunctionType.Exp)
                sumx = wrk.tile([1, B], F32, tag="sumx", name="sumx")
                nc.vector.tensor_reduce(sumx[:], exv, mybir.AxisListType.X,
                                        mybir.AluOpType.add)
                rc = wrk.tile([1, B], F32, tag="rc", name="rc")
                nc.vector.reciprocal(rc[:], sumx[:])
                wv = wrk.tile([1, BN], BF16, tag="wv", name="wv")
                nc.vector.scalar_tensor_tensor(
                    wv.rearrange("p (b n) -> p b n", n=N), exv, 1.0 / N,
                    rc[:, :, None].broadcast_to([1, B, N]),
                    op0=mybir.AluOpType.mult, op1=mybir.AluOpType.mult)

                # broadcast wv across partitions via K=1 matmul, then pool:
                wb_ps = []
                for h in range(2):
                    p = ps.tile([128, 512], F32, tag="ps", name=f"wb_ps{h}")
                    nc.tensor.matmul(p[:], ones1[:],
                                     wv[:, h * 512:(h + 1) * 512],
                                     start=True, stop=True)
                    wb_ps.append(p)
                imgp = []
                for dt in range(DT):
                    scr = wrk.tile([128, BN], F32, tag="poolscr", bufs=1,
                                   name=f"pscr{dt}")
                    for h in range(2):
                        nc.vector.tensor_mul(scr[:, h * 512:(h + 1) * 512],
                                             img[dt][:, h * 512:(h + 1) * 512],
                                             wb_ps[h][:])
                    t = per.tile([128, B], BF16, tag=f"imgp{dt}", name=f"imgp{dt}")
                    nc.vector.tensor_reduce(t[:],
                                            scr.rearrange("p (b n) -> p b n", n=N),
                                            mybir.AxisListType.X,
                                            mybir.AluOpType.add)
                    imgp.append(t)

                # ---- h1 = relu(Wh1 @ imgp + bh1) [D, B] ------------------
                h1 = []
                for m in range(DT):
                    p = ps.tile([128, B], F32, tag="ps", name=f"h1ps{m}")
                    for k in range(DT):
                        nc.tensor.matmul(p[:],
                                         wh1T_sb[k][:, m * 128:(m + 1) * 128],
                                         imgp[k][:],
                                         start=(k == 0), stop=(k == DT - 1))
                    t = per.tile([128, B], BF16, tag=f"h1{m}", name=f"h1{m}")
                    nc.scalar.activation(t[:], p[:],
                                         mybir.ActivationFunctionType.Relu,
                                         bias=bh1_sb[m][:])
                    h1.append(t)

                # ---- col-tiled bilinear: cols[32t+b, g] = out[b, g+16t] --
                def bilinear_phase(slabs, slab_src, pool, tag, stat, nm):
                    cols = wrk.tile([128, NG], F32, tag=f"{nm}cols",
                                    name=f"{nm}cols")
                    for g in range(NG):
                        gs = []
                        for t in range(4):
                            o = g + NG * t
                            if o not in slabs:
                                t_ = pool.tile([128, DT * D], F8, tag=tag,
                                               name=f"{tag}{o}")
                                nc.sync.dma_start(t_[:], slab_src[o])
                                slabs[o] = t_
                            gs.append(slabs[o])
                        p = ps.tile([128, D], F32, tag="ps", name=f"{nm}ps{g}")
                        for it in range(DT):
                            for t in range(4):
                                nc.tensor.matmul(
                                    p[32 * t:32 * t + B, :],
                                    stat[it][:],
                                    gs[t][:, it * D:(it + 1) * D],
                                    start=(it == 0), stop=(it == DT - 1),
                                    tile_position=(0, 32 * t))
                        scr = wrk.tile([128, D], F32, tag="bl_scr", bufs=2,
                                       name=f"{nm}scr{g}")
                        nc.vector.scalar_tensor_tensor(
                            scr[:], p[:], DESCB, textcT4[:],
                            op0=mybir.AluOpType.mult, op1=mybir.AluOpType.mult,
                            accum_out=cols[:, g:g + 1])
                    pT = ps.tile([NG, 128], F32, tag="ps", name=f"{nm}colsT")
                    nc.tensor.transpose(pT[:], cols[:], ident[:, :])
                    return pT

                # z1 phase: sg_sl = sigmoid(z1 + bz1) [OS, B] bf16
                zT = bilinear_phase(wz_slabs, wz1, wzp, "wz", imgp, "z1")
                ag_in = dp.tile([OS, B], BF16, tag="ag_in", name="ag_in")
                ag_out = dp.tile([D, B], BF16, tag="ag_out", name="ag_out",
                                 addr_space=shared_addr)
                for t in range(4):
                    sg_t = wrk.tile([NG, B], BF16, tag=f"sg{t}", name=f"sg{t}")
                    nc.scalar.activation(
                        sg_t[:], zT[:, 32 * t:32 * t + B],
                        mybir.ActivationFunctionType.Sigmoid,
                        bias=bz1_sb[t][:])
                    nc.scalar.dma_start(ag_in[NG * t:NG * (t + 1), :], sg_t[:])
                if use_collectives:
                    nc.gpsimd.collective_compute(
                        "AllGather", mybir.AluOpType.bypass, replica_groups=groups,
                        ins=[ag_in.opt()], outs=[ag_out.opt()])
                else:
                    for r in range(NCORES):
                        nc.sync.dma_start(ag_out[r * OS:(r + 1) * OS, :], ag_in[:])

                # gate = sg * h1 ; o1 = relu(Wo1 @ gate + bo1) [D, B]
                gate = []
                for dt in range(DT):
                    g = wrk.tile([128, B], BF16, tag=f"gate{dt}", name=f"gate{dt}")
                    nc.scalar.dma_start(g[:], ag_out[dt * 128:(dt + 1) * 128, :])
                    nc.vector.tensor_mul(g[:], g[:], h1[dt][:])
                    gate.append(g)
                o1 = []
                for m in range(DT):
                    p = ps.tile([128, B], F32, tag="ps", name=f"o1ps{m}")
                    for k in range(DT):
                        nc.tensor.matmul(p[:],
                                         wo1T_sb[k][:, m * 128:(m + 1) * 128],
                                         gate[k][:],
                                         start=(k == 0), stop=(k == DT - 1))
                    t = per.tile([128, B], BF16, tag=f"o1_{m}", name=f"o1_{m}")
                    nc.scalar.activation(t[:], p[:],
                                         mybir.ActivationFunctionType.Relu,
                                         bias=bo1_sb[m][:])
                    o1.append(t)

                # ---- post-fusion + classifier partial --------------------
                fT = bilinear_phase(wpf_slabs, wpf, wpfp, "wpf", o1, "pf")
                cp = ps.tile([1, B], F32, tag="ps", name="cp")
                for t in range(4):
                    fr_t = wrk.tile([NG, B], BF16, tag=f"fr{t}", name=f"fr{t}")
                    nc.scalar.activation(
                        fr_t[:], fT[:, 32 * t:32 * t + B],
                        mybir.ActivationFunctionType.Relu,
                        bias=bpf_sb[t][:])
                    nc.tensor.matmul(cp[:], wclsT_sb[t][:], fr_t[:],
                                     start=(t == 0), stop=(t == 3))
                osb = wrk.tile([1, B], F32, tag="osb", name="osb")
                nc.vector.tensor_copy(osb[:], cp[:])
                nc.scalar.dma_start(out_partial[:], osb[:])

            if reps > 1 and not use_collectives:
                with tc.For_i(0, reps, 1):
                    emit_body()
            else:
                for _ in range(reps):
                    emit_body()

    nc.compile()
    return nc


_NC_CACHE = {}


def _get_nc():
    if "nc" not in _NC_CACHE:
        _NC_CACHE["nc"] = build_nc()
    return _NC_CACHE["nc"]


def make_in_maps(inputs):
    """Host-side sharding + fp8/bf16 packing into 8 per-core input maps."""
    ii = {k: np.asarray(v, dtype=np.float32) for k, v in inputs.items()}
    X = ii["image_features"].reshape(BN, F)
    Xq = (X * SX).astype(NP_F8)                       # [BN, F]
    Wicq = (ii["W_ic"] * SW).astype(NP_F8)            # [D, F]
    Wz1q = (ii["Wz1"] * SW).astype(NP_F8)             # [D, D, D]
    Wpfq = (ii["Wpf"].reshape(D, D, D) * SW).astype(NP_F8)

    def pack_k(a2d, n):
        # [FS, n] f-major -> [KC, 128, 2*n] (2 k-tiles per chunk)
        return np.ascontiguousarray(
            a2d.reshape(KC, 2, 128, n).transpose(0, 2, 1, 3).reshape(
                KC, 128, 2 * n))

    def pack_slab(a3d):
        # [OS, 512, 512] -> [OS, 128, DT*512] (contraction tiled over part.)
        return np.ascontiguousarray(
            a3d.reshape(OS, DT, 128, D).transpose(0, 2, 1, 3).reshape(
                OS, 128, DT * D))

    bf = lambda a: np.ascontiguousarray(a).astype(NP_BF16)
    shared = {
        "textT": bf(ii["text_features"].T),
        "waT": bf(ii["Wa"].T),
        "wbT": bf(ii["Wb"].T),
        "wcT": bf(ii["Wc"].T),
        "wtT": bf(ii["Wt"].T),
        "wh1T": bf(ii["Wh1"].T),
        "wo1T": bf(ii["Wo1"].T),
        "b_ic": ii["b_ic"].reshape(D, 1),
        "ba": ii["ba"].reshape(A, 1),
        "bb": ii["bb"].reshape(A, 1),
        "bt": ii["bt"].reshape(D, 1),
        "bh1": ii["bh1"].reshape(D, 1),
        "bo1": ii["bo1"].reshape(D, 1),
    }
    in_maps = []
    for c in range(NCORES):
        fs = slice(c * FS, (c + 1) * FS)
        os_ = slice(c * OS, (c + 1) * OS)
        m = dict(shared)
        m["xT"] = pack_k(np.ascontiguousarray(Xq[:, fs].T), BN)
        m["wicT"] = pack_k(np.ascontiguousarray(Wicq[:, fs].T), D)
        m["wz1"] = pack_slab(Wz1q[os_])
        m["wpf"] = pack_slab(Wpfq[os_])
        m["wclsT"] = bf(ii["Wcls"][0, os_].reshape(OS, 1))
        m["bz1"] = np.ascontiguousarray(ii["bz1"][os_].reshape(OS, 1))
        m["bpf"] = np.ascontiguousarray(ii["bpf"][os_].reshape(OS, 1))
        in_maps.append(m)
    return in_maps


def gather_output(results, bcls):
    acc = np.zeros((1, B), np.float64)
    for c in range(NCORES):
        acc += results[c]["out_partial"].astype(np.float64)
    return (acc.T + bcls.astype(np.float64)).astype(np.float32)


def kernel(**inputs) -> np.ndarray:
    from concourse.bass_utils import run_bass_kernel_spmd

    nc = _get_nc()
    in_maps = make_in_maps(inputs)
    res = run_bass_kernel_spmd(nc, in_maps, list(range(NCORES)))
    return gather_output(res.results, np.asarray(inputs["bcls"], np.float32))


# revision 25
# speedup vs baseline: 1.6230x; 1.6230x over previous
"""Trainium2 Bass kernel for ConcreteAttentionModel (dense_mlp).

Model (reference):
  img = relu(einsum('bnf,df->bnd', image_features, W_ic) + b_ic)   B=16,N=64,F=65536,D=512
  gated attention-MIL pooling over patches -> img [B, D]
  text = relu(text @ Wt.T + bt)                                    [B, D]
  h1 = relu(img @ Wh1.T + bh1)
  z1 = einsum('bi,oij,bj->bo', img, Wz1, text) + bz1               Wz1 [D,D,D]
  o1 = relu((sigmoid(z1) * h1) @ Wo1.T + bo1)
  fused = relu(kron(o1, text) @ Wpf.T + bpf)                       Wpf [D, D*D]
  out = fused @ Wcls.T + bcls                                      [B, 1]

Sharding over 8 cores (v2):
  - Stage 1 sharded over the contraction dim F (8192 per core); fp32 partial
    [D, B*N] AllReduce'd on-device; attention pooling replicated per core.
  - Bilinear Wz1 and post-fusion Wpf sharded over their output dim (64 rows
    per core).

v2 perf scheme:
  - The four big tensors (X, W_ic, Wz1, Wpf) are cast to fp8e4m3 on the host
    (X*16, weights*512; descale folded into activation scale / STT scalar).
    Per-core HBM traffic drops ~193 MB -> ~56 MB.  Numpy-validated rel err
    ~1.1e-2 vs the 2e-2 gate.  All activations are bf16; PSUM stays fp32.
  - Bilinears are col-tiled 4x via tile_position: col-group t computes
    o = g + 16t, quadrupling PE utilization (stationary has only B=16 cols).
    The per-group PSUM bank [128, D] is drained by ONE [128,512] DVE STT
    against a 4x-replicated textcT, accumulating z1/fused columns for 4 o's
    at once (partitions 32t+b).
  - All wz1 slab DMAs + a wpf prefetch window are emitted at the head of the
    SP DMA ring so they stream during stage 1 (HWDGE launches are FIFO per
    engine).  Aux DMAs (collective bounce, gate/out) ride the ACT ring.
"""

import numpy as np
import ml_dtypes

import concourse.bass as bass
import concourse.mybir as mybir
import concourse.tile as tile
from concourse import bacc
from concourse.masks import make_identity

F32 = mybir.dt.float32
F32R = mybir.dt.float32r
BF16 = mybir.dt.bfloat16
F8 = mybir.dt.float8e4

NP_BF16 = ml_dtypes.bfloat16
NP_F8 = ml_dtypes.float8_e4m3

NCORES = 8
B, N, F = 16, 64, 65536
BN = B * N                      # 1024
D, A, T = 512, 256, 18
FS = F // NCORES                # 8192 per-core contraction slice
KT = FS // 128                  # 64 k-tiles
KC = KT // 2                    # 32 k-chunks (2 k-tiles per DMA)
OS = D // NCORES                # 64 output rows per core (bilinear/post-fusion)
DT = D // 128                   # 4 partition tiles of the D dim
AT = A // 128                   # 2 partition tiles of the A dim
NG = OS // 4                    # 16 col-tiled bilinear groups (4 o's each)

SX = 16.0                       # image fp8 scale
SW = 512.0                      # weight fp8 scale
DESC1 = 1.0 / (SX * SW)         # stage-1 descale
DESCB = 1.0 / SW                # bilinear descale

WZ_PRE = 50                     # wz1 slabs prefetched at the DMA-ring head
WPF_PRE = 16                    # wpf slabs prefetched at the DMA-ring head


def build_nc(use_collectives: bool = True, reps: int = 1,
             doublerow: bool = True, ar_bf16: bool = True,
             split_ar: bool = False):
    nc = bacc.Bacc("TRN2", target_bir_lowering=False, debug=False,
                   num_devices=NCORES)

    def dram(name, shape, dt=F32):
        return nc.dram_tensor(name, shape, dt, kind="ExternalInput").ap()

    # per-core sharded inputs (fp8, host-prepacked for contiguous DMA lines)
    xT = dram("xT", [KC, 128, 2 * BN], F8)
    wicT = dram("wicT", [KC, 128, 2 * D], F8)
    wz1 = dram("wz1", [OS, 128, DT * D], F8)   # o-slab [128, it, j]
    wpf = dram("wpf", [OS, 128, DT * D], F8)   # m-slab [128, ot, j]
    wclsT = dram("wclsT", [OS, 1], BF16)
    bz1 = dram("bz1", [OS, 1])
    bpf = dram("bpf", [OS, 1])
    # replicated inputs
    textT = dram("textT", [T, B], BF16)
    waT = dram("waT", [D, A], BF16)
    wbT = dram("wbT", [D, A], BF16)
    wcT = dram("wcT", [A, 1], BF16)
    wtT = dram("wtT", [T, D], BF16)
    wh1T = dram("wh1T", [D, D], BF16)
    wo1T = dram("wo1T", [D, D], BF16)
    b_ic = dram("b_ic", [D, 1])
    ba = dram("ba", [A, 1])
    bb = dram("bb", [A, 1])
    bt = dram("bt", [D, 1])
    bh1 = dram("bh1", [D, 1])
    bo1 = dram("bo1", [D, 1])

    out_partial = nc.dram_tensor("out_partial", [1, B], F32,
                                 kind="ExternalOutput").ap()

    groups = [list(range(NCORES))]

    with tile.TileContext(nc) as tc:
        with (
            nc.allow_low_precision(reason="fp8/bf16 matmul operands"),
            tc.tile_pool(name="const", bufs=1) as cst,
            tc.tile_pool(name="xk", bufs=3) as xk,
            tc.tile_pool(name="wk", bufs=3) as wk,
            tc.tile_pool(name="wz", bufs=WZ_PRE) as wzp,
            tc.tile_pool(name="wpfp", bufs=WPF_PRE) as wpfp,
            tc.tile_pool(name="work", bufs=1) as wrk,
            tc.tile_pool(name="persist", bufs=1) as per,
            tc.tile_pool(name="ps", bufs=8, space="PSUM") as ps,
            tc.tile_pool(name="dramp", bufs=1, space="DRAM") as dp,
        ):
            # ---- constants / small weights -------------------------------
            ident = cst.tile([128, 128], F32, tag="ident", name="ident")
            make_identity(nc, ident)
            ones_f = cst.tile([1, 128], F32, tag="ones_f", name="ones_f")
            nc.gpsimd.memset(ones_f[:], 1.0)
            ones1 = cst.tile([1, 128], BF16, tag="ones1", name="ones1")
            nc.vector.tensor_copy(ones1[:], ones_f[:])

            def load_const(src, shape, tag, dt=F32):
                t = cst.tile(shape, dt, tag=tag, name=tag)
                nc.sync.dma_start(t[:], src)
                return t

            textT_sb = load_const(textT[:], [T, B], "textT", BF16)
            wtT_sb = load_const(wtT[:], [T, D], "wtT", BF16)
            waT_sb = [load_const(waT[k * 128:(k + 1) * 128, :], [128, A],
                                 f"waT{k}", BF16) for k in range(DT)]
            wbT_sb = [load_const(wbT[k * 128:(k + 1) * 128, :], [128, A],
                                 f"wbT{k}", BF16) for k in range(DT)]
            wcT_sb = [load_const(wcT[k * 128:(k + 1) * 128, :], [128, 1],
                                 f"wcT{k}", BF16) for k in range(AT)]
            wh1T_sb = [load_const(wh1T[k * 128:(k + 1) * 128, :], [128, D],
                                  f"wh1T{k}", BF16) for k in range(DT)]
            wo1T_sb = [load_const(wo1T[k * 128:(k + 1) * 128, :], [128, D],
                                  f"wo1T{k}", BF16) for k in range(DT)]
            wclsT_sb = [load_const(wclsT[NG * t:NG * (t + 1), :], [NG, 1],
                                   f"wclsT{t}", BF16) for t in range(4)]
            bic_sb = [load_const(b_ic[k * 128:(k + 1) * 128, :], [128, 1], f"bic{k}")
                      for k in range(DT)]
            ba_sb = [load_const(ba[k * 128:(k + 1) * 128, :], [128, 1], f"ba{k}")
                     for k in range(AT)]
            bb_sb = [load_const(bb[k * 128:(k + 1) * 128, :], [128, 1], f"bb{k}")
                     for k in range(AT)]
            bt_sb = [load_const(bt[k * 128:(k + 1) * 128, :], [128, 1], f"bt{k}")
                     for k in range(DT)]
            bh1_sb = [load_const(bh1[k * 128:(k + 1) * 128, :], [128, 1], f"bh1{k}")
                      for k in range(DT)]
            bo1_sb = [load_const(bo1[k * 128:(k + 1) * 128, :], [128, 1], f"bo1{k}")
                      for k in range(DT)]
            bz1_sb = [load_const(bz1[NG * t:NG * (t + 1), :], [NG, 1], f"bz1{t}")
                      for t in range(4)]
            bpf_sb = [load_const(bpf[NG * t:NG * (t + 1), :], [NG, 1], f"bpf{t}")
                      for t in range(4)]

            def emit_body():
                # ---- DMA-ring head: prefetch launches -------------------
                # consumption order for both bilinears: group g uses
                # o = g + 16t for t = 0..3
                order = [g + NG * t for g in range(NG) for t in range(4)]
                wz_slabs = {}
                for o in order[:WZ_PRE]:
                    t_ = wzp.tile([128, DT * D], F8, tag="wz", name=f"wz{o}")
                    nc.sync.dma_start(t_[:], wz1[o])
                    wz_slabs[o] = t_
                wpf_slabs = {}
                for o in order[:WPF_PRE]:
                    t_ = wpfp.tile([128, DT * D], F8, tag="wpf", name=f"wpf{o}")
                    nc.sync.dma_start(t_[:], wpf[o])
                    wpf_slabs[o] = t_

                # ---- text branch: textc = relu(Wt @ text.T + bt) [D, B] --
                textc = []
                for m in range(DT):
                    p = ps.tile([128, B], F32, tag="ps", name=f"tc_ps{m}")
                    nc.tensor.matmul(p[:], wtT_sb[:, m * 128:(m + 1) * 128],
                                     textT_sb[:], start=True, stop=True)
                    t = per.tile([128, B], F32, tag=f"textc{m}", name=f"textc{m}")
                    nc.scalar.activation(t[:], p[:],
                                         mybir.ActivationFunctionType.Relu,
                                         bias=bt_sb[m][:])
                    textc.append(t)
                # textcT4 [128, D]: textcT4[32t+b, j] = textc[j, b], 4x repl.
                textcT4 = per.tile([128, D], F32, tag="textcT4", name="textcT4")
                for m in range(DT):
                    p = ps.tile([B, 128], F32, tag="ps", name=f"tct_ps{m}")
                    nc.tensor.transpose(p[:], textc[m][:], ident[:, :])
                    for t in range(4):
                        nc.vector.tensor_copy(
                            textcT4[32 * t:32 * t + B, m * 128:(m + 1) * 128],
                            p[:])

                # ---- stage 1: partial img.T = (W_ic @ X.T) slice ---------
                s1ps = [ps.tile([128, 512], F32, tag="ps", name=f"s1ps{i}")
                        for i in range(8)]
                for c in range(KC):
                    xt = xk.tile([128, 2 * BN], F8, tag="xk", name=f"x{c}")
                    nc.sync.dma_start(xt[:], xT[c])
                    wt = wk.tile([128, 2 * D], F8, tag="wk", name=f"w{c}")
                    nc.sync.dma_start(wt[:], wicT[c])
                    # fp8 DoubleRow: contract both k-tiles of the chunk at once
                    xt3 = xt.rearrange("p (k n) -> p k n", k=2)
                    wt3 = wt.rearrange("p (k n) -> p k n", k=2)
                    if doublerow:
                        first = c == 0
                        last = c == KC - 1
                        for dt in range(DT):
                            for h in range(2):
                                nc.tensor.matmul(
                                    s1ps[dt * 2 + h][:],
                                    wt3[:, :, dt * 128:(dt + 1) * 128],
                                    xt3[:, :, h * 512:(h + 1) * 512],
                                    start=first, stop=last,
                                    perf_mode=mybir.MatmulPerfMode.DoubleRow)
                    else:
                        for k in range(2):
                            first = c == 0 and k == 0
                            last = c == KC - 1 and k == 1
                            for dt in range(DT):
                                for h in range(2):
                                    nc.tensor.matmul(
                                        s1ps[dt * 2 + h][:],
                                        wt3[:, k, dt * 128:(dt + 1) * 128],
                                        xt3[:, k, h * 512:(h + 1) * 512],
                                        start=first, stop=last)

                # partial -> DRAM bounce -> AllReduce -> img tiles
                shared_addr = "Shared" if use_collectives else "Local"
                ARDT = BF16 if ar_bf16 else F32
                NH = 2 if split_ar else 1        # collective halves along D
                HD = DT // NH                    # d-tiles per half
                ar_in = [dp.tile([HD * 128, BN], ARDT, tag=f"ar_in{i}",
                                 name=f"ar_in{i}") for i in range(NH)]
                ar_out = [dp.tile([HD * 128, BN], ARDT, tag=f"ar_out{i}",
                                  name=f"ar_out{i}", addr_space=shared_addr)
                          for i in range(NH)]
                for dt in range(DT):
                    for h in range(2):
                        s = wrk.tile([128, 512], ARDT, tag="s1out", bufs=2,
                                     name=f"s1o{dt}{h}")
                        nc.vector.tensor_copy(s[:], s1ps[dt * 2 + h][:])
                        nc.scalar.dma_start(
                            ar_in[dt // HD][(dt % HD) * 128:(dt % HD + 1) * 128,
                                            h * 512:(h + 1) * 512],
                            s[:])
                for i in range(NH):
                    if use_collectives:
                        nc.gpsimd.collective_compute(
                            "AllReduce", mybir.AluOpType.add,
                            replica_groups=groups,
                            ins=[ar_in[i].opt()], outs=[ar_out[i].opt()])
                    else:
                        nc.sync.dma_start(ar_out[i][:], ar_in[i][:])

                # img = relu(sum * DESC1 + b_ic): [D, BN] as 4 bf16 tiles;
                # attention matmuls are k-outer so half-0 tiles are consumed
                # while half-1's AllReduce is still in flight.
                img = []
                for dt in range(DT):
                    raw = wrk.tile([128, BN], ARDT, tag="imgraw", bufs=1,
                                   name=f"imgraw{dt}")
                    nc.scalar.dma_start(
                        raw[:],
                        ar_out[dt // HD][(dt % HD) * 128:(dt % HD + 1) * 128, :])
                    t = per.tile([128, BN], BF16, tag=f"img{dt}", name=f"img{dt}")
                    nc.scalar.activation(t[:], raw[:],
                                         mybir.ActivationFunctionType.Relu,
                                         bias=bic_sb[dt][:], scale=DESC1)
                    img.append(t)

                # ---- attention: a=tanh(Wa@img+ba), g=sig(Wb@img+bb) ------
                ag_ps = {}
                for nm in ("a", "g"):
                    for m in range(AT):
                        for h in range(2):
                            ag_ps[nm, m, h] = ps.tile([128, 512], F32, tag="ps",
                                                      name=f"{nm}_ps{m}{h}")
                for k in range(DT):
                    for nm, wT_sb in (("a", waT_sb), ("g", wbT_sb)):
                        for m in range(AT):
                            for h in range(2):
                                nc.tensor.matmul(
                                    ag_ps[nm, m, h][:],
                                    wT_sb[k][:, m * 128:(m + 1) * 128],
                                    img[k][:, h * 512:(h + 1) * 512],
                                    start=(k == 0), stop=(k == DT - 1))
                a_sb, g_sb = [], []
                for nm, b_sb, func, outs in (
                        ("a", ba_sb, mybir.ActivationFunctionType.Tanh, a_sb),
                        ("g", bb_sb, mybir.ActivationFunctionType.Sigmoid, g_sb)):
                    for m in range(AT):
                        t = wrk.tile([128, BN], BF16, tag=f"{nm}{m}", name=f"{nm}{m}")
                        for h in range(2):
                            nc.scalar.activation(t[:, h * 512:(h + 1) * 512],
                                                 ag_ps[nm, m, h][:],
                                                 func, bias=b_sb[m][:])
                        outs.append(t)
                for m in range(AT):
                    nc.vector.tensor_mul(a_sb[m][:], a_sb[m][:], g_sb[m][:])

                # logits [1, BN] = Wc @ (a*g)
                sm = wrk.tile([1, BN], F32, tag="sm", name="sm")
                for h in range(2):
                    p = ps.tile([1, 512], F32, tag="ps", name=f"lg{h}")
                    for k in range(AT):
                        nc.tensor.matmul(p[:], wcT_sb[k][:],
                                         a_sb[k][:, h * 512:(h + 1) * 512],
                                         start=(k == 0), stop=(k == AT - 1))
                    nc.scalar.copy(sm[:, h * 512:(h + 1) * 512], p[:])

                # softmax over n (64) within each bag, * 1/N   -> wv [1, BN]
                smv = sm.rearrange("p (b n) -> p b n", n=N)
                mx = wrk.tile([1, B], F32, tag="mx", name="mx")
                nc.vector.tensor_reduce(mx[:], smv, mybir.AxisListType.X,
                                        mybir.AluOpType.max)
                ex = wrk.tile([1, BN], F32, tag="ex", name="ex")
                exv = ex.rearrange("p (b n) -> p b n", n=N)
                nc.vector.tensor_sub(exv, smv, mx[:, :, None].broadcast_to([1, B, N]))
                nc.scalar.activation(ex[:], ex[:], mybir.ActivationFunctionType.Exp)
                sumx = wrk.tile([1, B], F32, tag="sumx", name="sumx")
                nc.vector.tensor_reduce(sumx[:], exv, mybir.AxisListType.X,
                                        mybir.AluOpType.add)
                rc = wrk.tile([1, B], F32, tag="rc", name="rc")
                nc.vector.reciprocal(rc[:], sumx[:])
                wv = wrk.tile([1, BN], BF16, tag="wv", name="wv")
                nc.vector.scalar_tensor_tensor(
                    wv.rearrange("p (b n) -> p b n", n=N), exv, 1.0 / N,
                    rc[:, :, None].broadcast_to([1, B, N]),
                    op0=mybir.AluOpType.mult, op1=mybir.AluOpType.mult)

                # broadcast wv across partitions via K=1 matmul, then pool:
                wb_ps = []
                for h in range(2):
                    p = ps.tile([128, 512], F32, tag="ps", name=f"wb_ps{h}")
                    nc.tensor.matmul(p[:], ones1[:],
                                     wv[:, h * 512:(h + 1) * 512],
                                     start=True, stop=True)
                    wb_ps.append(p)
                imgp = []
                for dt in range(DT):
                    scr = wrk.tile([128, BN], F32, tag="poolscr", bufs=1,
                                   name=f"pscr{dt}")
                    for h in range(2):
                        nc.vector.tensor_mul(scr[:, h * 512:(h + 1) * 512],
                                             img[dt][:, h * 512:(h + 1) * 512],
                                             wb_ps[h][:])
                    t = per.tile([128, B], BF16, tag=f"imgp{dt}", name=f"imgp{dt}")
                    nc.vector.tensor_reduce(t[:],
                                            scr.rearrange("p (b n) -> p b n", n=N),
                                            mybir.AxisListType.X,
                                            mybir.AluOpType.add)
                    imgp.append(t)

                # ---- h1 = relu(Wh1 @ imgp + bh1) [D, B] ------------------
                h1 = []
                for m in range(DT):
                    p = ps.tile([128, B], F32, tag="ps", name=f"h1ps{m}")
                    for k in range(DT):
                        nc.tensor.matmul(p[:],
                                         wh1T_sb[k][:, m * 128:(m + 1) * 128],
                                         imgp[k][:],
                                         start=(k == 0), stop=(k == DT - 1))
                    t = per.tile([128, B], BF16, tag=f"h1{m}", name=f"h1{m}")
                    nc.scalar.activation(t[:], p[:],
                                         mybir.ActivationFunctionType.Relu,
                                         bias=bh1_sb[m][:])
                    h1.append(t)

                # ---- col-tiled bilinear: cols[32t+b, g] = out[b, g+16t] --
                def bilinear_phase(slabs, slab_src, pool, tag, stat, nm):
                    cols = wrk.tile([128, NG], F32, tag=f"{nm}cols",
                                    name=f"{nm}cols")
                    for g in range(NG):
                        gs = []
                        for t in range(4):
                            o = g + NG * t
                            if o not in slabs:
                                t_ = pool.tile([128, DT * D], F8, tag=tag,
                                               name=f"{tag}{o}")
                                nc.sync.dma_start(t_[:], slab_src[o])
                                slabs[o] = t_
                            gs.append(slabs[o])
                        p = ps.tile([128, D], F32, tag="ps", name=f"{nm}ps{g}")
                        for it in range(DT):
                            for t in range(4):
                                nc.tensor.matmul(
                                    p[32 * t:32 * t + B, :],
                                    stat[it][:],
                                    gs[t][:, it * D:(it + 1) * D],
                                    start=(it == 0), stop=(it == DT - 1),
                                    tile_position=(0, 32 * t))
                        scr = wrk.tile([128, D], F32, tag="bl_scr", bufs=2,
                                       name=f"{nm}scr{g}")
                        nc.vector.scalar_tensor_tensor(
                            scr[:], p[:], DESCB, textcT4[:],
                            op0=mybir.AluOpType.mult, op1=mybir.AluOpType.mult,
                            accum_out=cols[:, g:g + 1])
                    pT = ps.tile([NG, 128], F32, tag="ps", name=f"{nm}colsT")
                    nc.tensor.transpose(pT[:], cols[:], ident[:, :])
                    return pT

                # z1 phase: sg_sl = sigmoid(z1 + bz1) [OS, B] bf16
                zT = bilinear_phase(wz_slabs, wz1, wzp, "wz", imgp, "z1")
                ag_in = dp.tile([OS, B], BF16, tag="ag_in", name="ag_in")
                ag_out = dp.tile([D, B], BF16, tag="ag_out", name="ag_out",
                                 addr_space=shared_addr)
                for t in range(4):
                    sg_t = wrk.tile([NG, B], BF16, tag=f"sg{t}", name=f"sg{t}")
                    nc.scalar.activation(
                        sg_t[:], zT[:, 32 * t:32 * t + B],
                        mybir.ActivationFunctionType.Sigmoid,
                        bias=bz1_sb[t][:])
                    nc.scalar.dma_start(ag_in[NG * t:NG * (t + 1), :], sg_t[:])
                if use_collectives:
                    nc.gpsimd.collective_compute(
                        "AllGather", mybir.AluOpType.bypass, replica_groups=groups,
                        ins=[ag_in.opt()], outs=[ag_out.opt()])
                else:
                    for r in range(NCORES):
                        nc.sync.dma_start(ag_out[r * OS:(r + 1) * OS, :], ag_in[:])

                # gate = sg * h1 ; o1 = relu(Wo1 @ gate + bo1) [D, B]
                gate = []
                for dt in range(DT):
                    g = wrk.tile([128, B], BF16, tag=f"gate{dt}", name=f"gate{dt}")
                    nc.scalar.dma_start(g[:], ag_out[dt * 128:(dt + 1) * 128, :])
                    nc.vector.tensor_mul(g[:], g[:], h1[dt][:])
                    gate.append(g)
                o1 = []
                for m in range(DT):
                    p = ps.tile([128, B], F32, tag="ps", name=f"o1ps{m}")
                    for k in range(DT):
                        nc.tensor.matmul(p[:],
                                         wo1T_sb[k][:, m * 128:(m + 1) * 128],
                                         gate[k][:],
                                         start=(k == 0), stop=(k == DT - 1))
                    t = per.tile([128, B], BF16, tag=f"o1_{m}", name=f"o1_{m}")
                    nc.scalar.activation(t[:], p[:],
                                         mybir.ActivationFunctionType.Relu,
                                         bias=bo1_sb[m][:])
                    o1.append(t)

                # ---- post-fusion + classifier partial --------------------
                fT = bilinear_phase(wpf_slabs, wpf, wpfp, "wpf", o1, "pf")
                cp = ps.tile([1, B], F32, tag="ps", name="cp")
                for t in range(4):
                    fr_t = wrk.tile([NG, B], BF16, tag=f"fr{t}", name=f"fr{t}")
                    nc.scalar.activation(
                        fr_t[:], fT[:, 32 * t:32 * t + B],
                        mybir.ActivationFunctionType.Relu,
                        bias=bpf_sb[t][:])
                    nc.tensor.matmul(cp[:], wclsT_sb[t][:], fr_t[:],
                                     start=(t == 0), stop=(t == 3))
                osb = wrk.tile([1, B], F32, tag="osb", name="osb")
                nc.vector.tensor_copy(osb[:], cp[:])
                nc.scalar.dma_start(out_partial[:], osb[:])

            if reps > 1 and not use_collectives:
                with tc.For_i(0, reps, 1):
                    emit_body()
            else:
                for _ in range(reps):
                    emit_body()

    nc.compile()
    return nc


_NC_CACHE = {}


def _get_nc():
    if "nc" not in _NC_CACHE:
        _NC_CACHE["nc"] = build_nc()
    return _NC_CACHE["nc"]


def make_in_maps(inputs):
    """Host-side sharding + fp8/bf16 packing into 8 per-core input maps."""
    ii = {k: np.asarray(v, dtype=np.float32) for k, v in inputs.items()}
    X = ii["image_features"].reshape(BN, F)
    Xq = (X * SX).astype(NP_F8)                       # [BN, F]
    Wicq = (ii["W_ic"] * SW).astype(NP_F8)            # [D, F]
    Wz1q = (ii["Wz1"] * SW).astype(NP_F8)             # [D, D, D]
    Wpfq = (ii["Wpf"].reshape(D, D, D) * SW).astype(NP_F8)

    def pack_k(a2d, n):
        # [FS, n] f-major -> [KC, 128, 2*n] (2 k-tiles per chunk)
        return np.ascontiguousarray(
            a2d.reshape(KC, 2, 128, n).transpose(0, 2, 1, 3).reshape(
                KC, 128, 2 * n))

    def pack_slab(a3d):
        # [OS, 512, 512] -> [OS, 128, DT*512] (contraction tiled over part.)
        return np.ascontiguousarray(
            a3d.reshape(OS, DT, 128, D).transpose(0, 2, 1, 3).reshape(
                OS, 128, DT * D))

    bf = lambda a: np.ascontiguousarray(a).astype(NP_BF16)
    shared = {
        "textT": bf(ii["text_features"].T),
        "waT": bf(ii["Wa"].T),
        "wbT": bf(ii["Wb"].T),
        "wcT": bf(ii["Wc"].T),
        "wtT": bf(ii["Wt"].T),
        "wh1T": bf(ii["Wh1"].T),
        "wo1T": bf(ii["Wo1"].T),
        "b_ic": ii["b_ic"].reshape(D, 1),
        "ba": ii["ba"].reshape(A, 1),
        "bb": ii["bb"].reshape(A, 1),
        "bt": ii["bt"].reshape(D, 1),
        "bh1": ii["bh1"].reshape(D, 1),
        "bo1": ii["bo1"].reshape(D, 1),
    }
    in_maps = []
    for c in range(NCORES):
        fs = slice(c * FS, (c + 1) * FS)
        os_ = slice(c * OS, (c + 1) * OS)
        m = dict(shared)
        m["xT"] = pack_k(np.ascontiguousarray(Xq[:, fs].T), BN)
        m["wicT"] = pack_k(np.ascontiguousarray(Wicq[:, fs].T), D)
        m["wz1"] = pack_slab(Wz1q[os_])
        m["wpf"] = pack_slab(Wpfq[os_])
        m["wclsT"] = bf(ii["Wcls"][0, os_].reshape(OS, 1))
        m["bz1"] = np.ascontiguousarray(ii["bz1"][os_].reshape(OS, 1))
        m["bpf"] = np.ascontiguousarray(ii["bpf"][os_].reshape(OS, 1))
        in_maps.append(m)
    return in_maps


def gather_output(results, bcls):
    acc = np.zeros((1, B), np.float64)
    for c in range(NCORES):
        acc += results[c]["out_partial"].astype(np.float64)
    return (acc.T + bcls.astype(np.float64)).astype(np.float32)


def kernel(**inputs) -> np.ndarray:
    from concourse.bass_utils import run_bass_kernel_spmd

    nc = _get_nc()
    in_maps = make_in_maps(inputs)
    res = run_bass_kernel_spmd(nc, in_maps, list(range(NCORES)))
    return gather_output(res.results, np.asarray(inputs["bcls"], np.float32))
